# revision 22
# baseline (speedup 1.0000x reference)
"""Trainium2 Bass kernel for the DLI (dialogue-turn ordering) loss — v2.

Math (exact reduction of the reference):
  With 2 classes, NLL(label y) = softplus(l_{1-y} - l_y).
  u[b,j] = enc[b,j] @ (W[:D,1]-W[:D,0]),
  v[b,k] = enc[b,k] @ (W[D:,1]-W[D:,0]),
  c      = b[1]-b[0],  d[b,j,k] = u[b,j] + v[b,k] + c
  label = 1 iff k == j-1; valid pairs: k < j < len_b;  softplus(-d) = softplus(d) - d
  =>  sum_nll = sum_{valid} softplus(d) - sum_{b, 1<=j<len_b} d[b,j,j-1]
  loss = sum_nll / max(n_valid, 1)

v2 layout (ragged packing + PE dots):
  Only rows j < len_b ever matter (arch is ragged_sequence).  The host
  length-balances the 64 batches into 8 cores of 8, packs each core's valid
  rows contiguously, and ships them TRANSPOSED: encT [D=2048, R] f32 where
  R = packed rows padded to a multiple of 128 (512 for the expected data).
  HBM traffic is halved vs the full [8, 128, 2048] and the transposed layout
  lets the TensorEngine do the dots:

    psum_uv[2, R] += wuvT_chunk[128, 2].T @ encT_chunk[128, R]   (16 chunks)

  as f32r matmuls (1 cycle/row at R >= 256), replacing the DVE/ACT dot
  pipeline of v1 that was the bottleneck (ACT 30us + DVE 30us busy).  All
  DMAs are plain-dtype HWDGE on the sync queue (no SWDGE => no GpSimd
  descriptor work, no dge drains).

  Phase B on packed coordinates: tiles of 128 rows; pair blocks (a,a) and
  (a,a-1) only (a batch spans at most 2 tiles).  d is built additively in
  PSUM by rank-1 fp16 matmuls:
    diag:  ident@triNEG + ones@v16 + ones@bp16      (tri kills k>=j)
    off :  ones@v16 + ones@bp16                     (all global k < j)
  where bp16[k] = +A*bid[k] (A=512, fp16-exact) or NEG for padding, and the
  Exp bias column carries u[j] + c - A*bid[j] in f32 (via a K=1 PE
  transpose of the u row).  Cross-batch pairs get exp(d - A*dbid) -> 0;
  same-batch pairs cancel the A terms exactly.  Exp/Ln(1+x) on ACT with
  fused row-sum accumulation; label-1 diagonal handled as masked row sums
  of raw u,v (masks shipped negated so the final PSUM matmul accumulates
  the subtraction).  Host divides by exact n_valid and fixes the c terms.
"""

import glob
import json
import os
import shutil
import sys
import tempfile

if "/opt/trn_rl_repo" not in sys.path:
    sys.path.insert(0, "/opt/trn_rl_repo")


def _force_combined_act_table():
    """Point walrus at an act_info.json holding only natural_log_exp_and_others
    (contains exp+ln), so every ACTIVATE shares one table."""
    if os.environ.get("BASS_ACT_ROOT_JSON_PATH"):
        return
    from neuronxcc.driver.Job import Job  # type: ignore

    pwp = None
    for cand in glob.glob(os.path.join(Job.getPackageDir(), "pwp", "pwp_bin_*")):
        if os.path.exists(os.path.join(cand, "act_info.json")):
            pwp = cand
            break
    if pwp is None:
        return
    info = json.load(open(os.path.join(pwp, "act_info.json")))
    keep = [t for t in info.get("act_func_sets", [])
            if t.get("name") == "natural_log_exp_and_others"]
    if not keep:
        return
    out_dir = os.path.join(tempfile.gettempdir(), "dli_act_combined")
    os.makedirs(out_dir, exist_ok=True)
    for t in keep:
        for k in info.get("pwp_file_keys", []):
            f = t.get(k)
            src = os.path.join(pwp, f) if f else None
            if src and os.path.exists(src):
                dst = os.path.join(out_dir, f)
                if not os.path.exists(dst):
                    shutil.copy(src, dst)
    info = dict(info)
    info["act_func_sets"] = keep
    with open(os.path.join(out_dir, "act_info.json"), "w") as f:
        json.dump(info, f)
    os.environ["BASS_ACT_ROOT_JSON_PATH"] = os.path.join(out_dir, "act_info.json")


_force_combined_act_table()

from contextlib import ExitStack

import numpy as np

import concourse.bacc as bacc
import concourse.bass as bass
import concourse.hw_specs as hw_specs
import concourse.mybir as mybir
import concourse.tile as tile

# Make bass's act-table placement agree with the trimmed act_info.json walrus
# sees: only the combined exp+ln table exists, so every ACTIVATE maps to
# act_func_set_id 0 and the table is loaded exactly once.
_orig_get_act_tables = hw_specs.get_activation_tables


def _combined_act_tables(module_arch):
    tabs = _orig_get_act_tables(module_arch)
    kept = {k: v for k, v in tabs.items() if k == "natural_log_exp_and_others"}
    return kept if kept and os.environ.get("BASS_ACT_ROOT_JSON_PATH") else tabs


hw_specs.get_activation_tables = _combined_act_tables
bacc.get_activation_tables = _combined_act_tables

# Cheaper kernel teardown: drain + one all-engine barrier + sem clear. The
# stock epilogue adds a second all-engine barrier after the clear; engines
# that pass the first barrier only run their terminal branch, and the next
# execution starts only after every engine (incl. the clearing one) halts,
# so the second barrier only adds ~4us of EVSEM latency.
from concourse.vector_clock import ScopedClock as _ScopedClock


def _cheap_drain_and_barrier(self, tick_clock, wait_clock):
    drain_inst = self.nc.sync.drain()
    wait_clock.add_sem_waits(
        drain_inst.ins, _ScopedClock({None: tick_clock.global_clock})
    )
    self.nc.all_engine_barrier()
    popped = self.nc._tile_sem_poison_stack.pop()
    assert popped is self._sem_poison
    self.nc.clear_and_free_semaphores(list(self.sems.allocated().values()))


tile.TileContext._drain_and_barrier = _cheap_drain_and_barrier

F32 = mybir.dt.float32
BF16 = mybir.dt.bfloat16
FP16 = mybir.dt.float16
ALU = mybir.AluOpType
ACTF = mybir.ActivationFunctionType
AXX = mybir.AxisListType.X

BSZ, L, D = 64, 128, 2048
N_CORES = 8
NB = BSZ // N_CORES  # batches per core
NCH = D // 128  # 16 contraction chunks
NEG = -30000.0  # additive suppression; exp(NEG + anything sane) == 0 in f32
ABID = 512.0  # batch-id suppression scale; ABID*bid is fp16-exact for bid<=7


def build_program(R_data, R_tile):
    """R_data = packed row count (mult of 32, >=256); R_tile = mult of 128."""
    T = R_tile // 128
    NBLK = 2 * T - 1
    W32 = 2 * NCH + 3 * T + 3  # wvu | auxc | mcols | ones_col | ident2
    nc = bacc.Bacc("TRN2", target_bir_lowering=False, debug=False, num_devices=1)

    enc = nc.dram_tensor("enc", [D, R_data], F32, kind="ExternalInput").ap()
    sf32 = nc.dram_tensor("sf32", [128, W32], F32, kind="ExternalInput").ap()
    # per-block additive masks (tri + batch-boundary + padding in one):
    # block order (0,0), (1,1), (1,0), (2,2), (2,1), ...
    sf16m = nc.dram_tensor("sf16m", [128, NBLK * 128], FP16, kind="ExternalInput").ap()
    sf16r = nc.dram_tensor("sf16r", [1, 128], FP16, kind="ExternalInput").ap()
    out = nc.dram_tensor("out", [1, 1], F32, kind="ExternalOutput").ap()

    # psum column slices of <=512 f32 (one 2KB bank each); only R_data cols
    slices = [(s, min(s + 512, R_data)) for s in range(0, R_data, 512)]

    # (kind, chunks): 'hw' = HWDGE f32 + DVE cast (fast start),
    # 'sw' = SWDGE casting DMA (GpSimd descriptor gen runs concurrently)
    GROUPS = ([('hw', 1), ('hw', 3), ('hw', 4), ('sw', 4), ('sw', 4)]
              if NCH == 16 else [('hw', NCH)])

    with tile.TileContext(nc) as tc, ExitStack() as ctx:
        consts = ctx.enter_context(tc.tile_pool(name="consts", bufs=1))
        accs = ctx.enter_context(tc.tile_pool(name="accs", bufs=1))
        enc_pool = ctx.enter_context(tc.tile_pool(name="enc", bufs=1))
        junk_pool = ctx.enter_context(tc.tile_pool(name="junk", bufs=2))
        psum_uv_pool = ctx.enter_context(tc.tile_pool(name="psuv", bufs=1, space="PSUM"))
        psum_d_pool = ctx.enter_context(tc.tile_pool(name="psd", bufs=2, space="PSUM"))
        psum_misc = ctx.enter_context(tc.tile_pool(name="psm", bufs=1, space="PSUM"))

        # ---- ACT table preload: dummy exp so the ~1.5us table load happens
        # during the DMA phase rather than before the first real Exp ----
        dummy = consts.tile([1, 1], F32)
        nc.gpsimd.memset(dummy[:], 0.0)
        dummy2 = consts.tile([1, 1], F32)
        nc.scalar.activation(dummy2[:], dummy[:], ACTF.Exp)

        # ---- enc group DMAs (single sync queue; each DMA_DIRECT2D trigger
        # costs ~750ns serially on the engine), constants interleaved ----
        encb, encf, ch_group = [], [], {}
        ch0 = 0
        sf32_sb = consts.tile([128, W32], F32)
        sf16m_sb = consts.tile([128, NBLK * 128], FP16)
        sf16r_sb = consts.tile([1, 128], FP16)
        for g, (kind, cpg) in enumerate(GROUPS):
            src = enc[128 * ch0 : 128 * (ch0 + cpg), :].rearrange(
                "(c p) r -> p c r", p=128)
            tb = enc_pool.tile([128, cpg * R_data], BF16, name=f"encb{g}", tag=f"encb{g}")
            if kind == 'sw':
                nc.gpsimd.dma_start(tb[:].rearrange("p (c r) -> p c r", c=cpg), src)
                encf.append(None)
            else:
                tf = enc_pool.tile([128, cpg * R_data], F32,
                                   name=f"encf{g}", tag=f"encf{g}")
                nc.sync.dma_start(tf[:].rearrange("p (c r) -> p c r", c=cpg), src)
                encf.append(tf)
            encb.append(tb)
            for cl in range(cpg):
                ch_group[ch0 + cl] = (g, cl)
            ch0 += cpg
            if g == 0:
                nc.sync.dma_start(sf32_sb[:], sf32[:])
                nc.sync.dma_start(sf16m_sb[:], sf16m[:])
                nc.sync.dma_start(sf16r_sb[:], sf16r[:])

        auxc_sb = sf32_sb[:, 2 * NCH : 2 * NCH + T]           # u bias addend cols
        mcols = sf32_sb[:, 2 * NCH + T : 2 * NCH + 3 * T]     # negated diag masks
        ones_col = sf32_sb[:, 2 * NCH + 3 * T : 2 * NCH + 3 * T + 1]
        ident2 = sf32_sb[0:2, 2 * NCH + 3 * T + 1 : 2 * NCH + 3 * T + 3]
        ones16 = sf16r_sb[0:1, 0:128]

        wvu_bf = consts.tile([128, 2 * NCH], BF16)
        nc.vector.tensor_copy(wvu_bf[:], sf32_sb[:, 0 : 2 * NCH])
        for ch in range(NCH):
            g, cl = ch_group[ch]
            if encf[g] is not None:
                nc.vector.tensor_copy(
                    encb[g][:, cl * R_data : (cl + 1) * R_data],
                    encf[g][:, cl * R_data : (cl + 1) * R_data])

        # ---- v,u dots on the PE (v row 0 so it is partition-0 readable) ----
        psum_uv = []
        for s, (c0, c1) in enumerate(slices):
            psum_uv.append(
                psum_uv_pool.tile([2, c1 - c0], F32, tag=f"uv{s}", name=f"uv{s}"))
        for ch in range(NCH):
            g, cl = ch_group[ch]
            for s, (c0, c1) in enumerate(slices):
                nc.tensor.matmul(
                    psum_uv[s][:], lhsT=wvu_bf[:, 2 * ch : 2 * ch + 2],
                    rhs=encb[g][:, cl * R_data + c0 : cl * R_data + c1],
                    start=(ch == 0), stop=(ch == NCH - 1),
                )

        # ---- extraction: v row directly; u columns (for the exp bias) via
        # small [2,128]->[128,2] PE transposes. A few junk matmuls keep the
        # PE p-state warm across the DVE extraction gap. ----
        v16 = accs.tile([1, R_tile], FP16)
        if R_data < R_tile:
            nc.vector.memset(v16[:, R_data:R_tile], 0.0)
        for s, (c0, c1) in enumerate(slices):
            nc.vector.tensor_copy(v16[:, c0:c1], psum_uv[s][0:1, :])
        uv_sb = accs.tile([2, R_tile], F32)
        if R_data < R_tile:
            nc.vector.memset(uv_sb[:, R_data:R_tile], 0.0)
        for s, (c0, c1) in enumerate(slices):
            nc.vector.tensor_copy(uv_sb[:, c0:c1], psum_uv[s][:])

        psum_j = psum_misc.tile([128, 128], F32, tag="junkp", name="junkp")
        for _ in range(3):
            nc.tensor.matmul(psum_j[:], lhsT=sf16m_sb[:, 0:128],
                             rhs=sf16m_sb[:, 0:128], start=True, stop=True)

        psum_t = psum_misc.tile([128, 2 * T], F32, tag="uc", name="uc")
        ubias = accs.tile([128, T], F32)
        for a in range(T):
            nc.tensor.matmul(
                psum_t[:, 2 * a : 2 * a + 2],
                lhsT=uv_sb[0:2, 128 * a : 128 * (a + 1)],
                rhs=ident2, is_transpose=True,
            )
            nc.vector.tensor_add(ubias[:, a : a + 1], psum_t[:, 2 * a + 1 : 2 * a + 2],
                                 auxc_sb[:, a : a + 1])

        # ---- phase B: per tile a, blocks (a,a)+(a,a-1) share one psum pair;
        # PE does only the v broadcast; DVE adds the host mask from psum;
        # ACT does one biased Exp per tile and grouped Ln(1+x) row-sums ----
        nblk_of = [1 if a == 0 else 2 for a in range(T)]
        ln_groups = []  # list of (exg_tile, [tiles]) widths 384/512/...
        RS = accs.tile([128, max(1, (T + 1) // 2)], F32)
        # pair tiles into ln groups of total width <= 512
        grp, wsum, groups = [], 0, []
        for a in range(T):
            w = 128 * nblk_of[a]
            if wsum + w > 512 and grp:
                groups.append(grp)
                grp, wsum = [], 0
            grp.append(a)
            wsum += w
        if grp:
            groups.append(grp)
        exg_tiles = []
        exg_of = {}
        for gi, g in enumerate(groups):
            wtot = sum(128 * nblk_of[a] for a in g)
            tgl = accs.tile([128, wtot], F32, tag=f"exg{gi}", name=f"exg{gi}")
            exg_tiles.append(tgl)
            off = 0
            for a in g:
                exg_of[a] = (gi, off)
                off += 128 * nblk_of[a]

        # label-1 diagonal in column space (DVE; overlaps phase B below)
        uvm = accs.tile([128, 2 * T], F32)
        nc.vector.tensor_mul(uvm[:], psum_t[:], mcols)
        dsumc = accs.tile([128, 1], F32)
        nc.vector.reduce_sum(dsumc[:], uvm[:], axis=AXX)

        blk_idx = 0
        for a in range(T):
            nb = nblk_of[a]
            pd = psum_d_pool.tile([128, 128 * nb], F32, tag="pd")
            for q in range(nb):
                b = a - q
                nc.tensor.matmul(
                    pd[:, 128 * q : 128 * (q + 1)], lhsT=ones16,
                    rhs=v16[:, 128 * b : 128 * (b + 1)], start=True, stop=True)
            exd = junk_pool.tile([128, 128 * nb], F32, tag="exd")
            nc.vector.tensor_add(
                exd[:], pd[:], sf16m_sb[:, 128 * blk_idx : 128 * (blk_idx + nb)])
            blk_idx += nb
            gi, off = exg_of[a]
            nc.scalar.activation(
                exg_tiles[gi][:, off : off + 128 * nb], exd[:], ACTF.Exp,
                bias=ubias[:, a : a + 1],
            )
            if a == groups[len(ln_groups)][-1]:
                gi2 = len(ln_groups)
                sp = junk_pool.tile(list(exg_tiles[gi2].shape), F32, tag="sp")
                nc.scalar.activation(sp[:], exg_tiles[gi2][:], ACTF.Ln, bias=1.0,
                                     accum_out=RS[:, gi2 : gi2 + 1])
                ln_groups.append(gi2)

        # ---- final reduction ----
        accA = accs.tile([128, 1], F32)
        nc.vector.reduce_sum(accA[:], RS[:, 0 : len(groups)], axis=AXX)
        nc.vector.tensor_add(accA[:], accA[:], dsumc[:])
        psum_s = psum_misc.tile([1, 1], F32, tag="psm", name="psm")
        nc.tensor.matmul(psum_s[:], lhsT=accA[:], rhs=ones_col,
                         start=True, stop=True)
        out_t = accs.tile([1, 1], F32)
        nc.vector.tensor_copy(out_t[:], psum_s[:])
        nc.sync.dma_start(out[:], out_t[:])

    nc.compile()
    return nc


_NC_CACHE = {}
_LAST_KEY = None


def _get_nc(key=None):
    global _LAST_KEY
    if key is None:
        key = _LAST_KEY if _LAST_KEY is not None else (544, 640)
    if key not in _NC_CACHE:
        _NC_CACHE[key] = build_program(*key)
    _LAST_KEY = key
    return _NC_CACHE[key]


def _prep(encoder_output, mask, W, b):
    """Host-side prep: length-balanced shard + packed transposed layout."""
    enc = np.asarray(encoder_output, dtype=np.float32)
    W = np.asarray(W, dtype=np.float32)
    b = np.asarray(b, dtype=np.float32).reshape(2)
    mask = np.asarray(mask)
    c = float(b[1] - b[0])
    lens = mask.astype(np.int64).sum(axis=1)  # [BSZ]

    # greedy length-balance into N_CORES groups of NB
    order = np.argsort(-lens, kind="stable")
    assign = [[] for _ in range(N_CORES)]
    loads = [0] * N_CORES
    for bidx in order:
        for cid in sorted(range(N_CORES), key=lambda q: (loads[q], q)):
            if len(assign[cid]) < NB:
                assign[cid].append(int(bidx))
                loads[cid] += int(lens[bidx])
                break
    R_data = max(256, -(-max(loads) // 64) * 64)
    R_tile = -(-R_data // 128) * 128
    T = R_tile // 128
    NBLK = 2 * T - 1
    W32 = 2 * NCH + 3 * T + 3

    wd = np.stack([W[D:, 1] - W[D:, 0], W[:D, 1] - W[:D, 0]])  # [v-w, u-w]

    maps = []
    for cid in range(N_CORES):
        buf = np.zeros((R_data, D), dtype=np.float32)
        addu = np.full(R_tile, NEG, dtype=np.float32)   # u-side bias addend
        m1n = np.zeros(R_tile, dtype=np.float32)
        m2n = np.zeros(R_tile, dtype=np.float32)
        bid = np.full(R_tile, -1, dtype=np.int64)       # batch id per row
        off = 0
        for beta, bidx in enumerate(assign[cid]):
            ln_ = int(lens[bidx])
            buf[off : off + ln_] = enc[bidx, :ln_]
            addu[off : off + ln_] = c
            bid[off : off + ln_] = beta
            m1n[off + 1 : off + ln_] = -1.0        # m1 negated (not first row)
            m2n[off : off + ln_ - 1] = -1.0        # m2 negated (not last row)
            off += ln_
        # per-block additive masks: 0 iff same batch AND global k < j
        gj = np.arange(R_tile)
        valid = (bid[:, None] == bid[None, :]) & (bid[:, None] >= 0) & (
            gj[None, :] < gj[:, None])
        sf16m = np.full((128, NBLK * 128), NEG, dtype=np.float16)
        blk = 0
        for a in range(T):
            for q in range(1 if a == 0 else 2):
                bt = a - q
                sub = valid[128 * a : 128 * (a + 1), 128 * bt : 128 * (bt + 1)]
                sf16m[:, 128 * blk : 128 * (blk + 1)] = np.where(sub, 0.0, NEG)
                blk += 1
        sf32 = np.zeros((128, W32), dtype=np.float32)
        sf32[:, 0 : 2 * NCH] = wd.T.reshape(NCH, 128, 2).transpose(
            1, 0, 2).reshape(128, 2 * NCH)
        sf32[:, 2 * NCH : 2 * NCH + T] = addu.reshape(T, 128).T
        # mcols matches psum_t layout: even col = v (m2n), odd col = u (m1n)
        mc = np.stack([m2n.reshape(T, 128).T, m1n.reshape(T, 128).T], axis=2)
        sf32[:, 2 * NCH + T : 2 * NCH + 3 * T] = mc.reshape(128, 2 * T)
        sf32[:, 2 * NCH + 3 * T] = 1.0             # ones_col
        sf32[0, 2 * NCH + 3 * T + 1] = 1.0         # ident2
        sf32[1, 2 * NCH + 3 * T + 2] = 1.0
        sf16r = np.ones((1, 128), dtype=np.float16)
        maps.append(
            {
                "enc": np.ascontiguousarray(buf.T),
                "sf32": sf32,
                "sf16m": sf16m,
                "sf16r": sf16r,
            }
        )
    diag_c = float(c * (lens - 1).sum())
    n_valid = int((lens * (lens - 1) // 2).sum())
    return maps, diag_c, n_valid, (R_data, R_tile)


def kernel(encoder_output, mask, W, b, _run_kwargs=None):
    from concourse.bass_utils import run_bass_kernel_spmd

    maps, diag_c, n_valid, key = _prep(np.asarray(encoder_output), mask, W, b)
    nc = _get_nc(key)
    res = run_bass_kernel_spmd(nc, maps, core_ids=list(range(N_CORES)),
                               **(_run_kwargs or {}))
    total = float(sum(np.float64(r["out"][0, 0]) for r in res.results))
    total -= diag_c
    loss = total / max(n_valid, 1)
    out = np.array(loss, dtype=np.float32)
    if _run_kwargs is not None:
        return out, res
    return out


# revision 24
# speedup vs baseline: 1.0310x; 1.0310x over previous
"""Trainium2 Bass kernel for the DLI (dialogue-turn ordering) loss — v8.

Math (exact reduction of the reference):
  With 2 classes, NLL(label y) = softplus(l_{1-y} - l_y).
  u[b,j] = enc[b,j] @ (W[:D,1]-W[:D,0]),
  v[b,k] = enc[b,k] @ (W[D:,1]-W[D:,0]),
  c      = b[1]-b[0],  d[b,j,k] = u[b,j] + v[b,k] + c
  label = 1 iff k == j-1; valid pairs: k < j < len_b;  softplus(-d) = softplus(d) - d
  =>  sum_nll = sum_{valid} softplus(d) - sum_{b, 1<=j<len_b} d[b,j,j-1]
  loss = sum_nll / max(n_valid, 1)

Ragged packing + PE dots (the arch is ragged_sequence — only rows j < len_b
matter, ~half of the 64MB input).  The host length-balances the 64 batches
into 8 cores of 8, packs each core's valid rows contiguously and ships them
TRANSPOSED: encT [D=2048, R_data] f32, R_data = rows rounded to 64 (576 for
the expected data; rows 256B-aligned for DMA).  HBM traffic drops ~45%, and
the transposed layout moves the u,v dots from DVE/ACT (the v1 bottleneck:
30us busy each) onto the idle TensorEngine:

    psum_uv[2, R] += wvuT_chunk[128, 2].T @ encT_chunk[128, R]   (16 chunks,
    bf16, PSUM f32 accumulation; v in row 0 so it is partition-0 readable)

enc arrives via SWDGE casting DMAs (HBM reads stay f32; bf16 lands in SBUF,
no staging or DVE casts), in groups of (1,1,2,4,4,4) chunks — the first
groups small so the dot pipeline starts as early as possible; each DMA
trigger costs ~750ns serially on its engine, so chunks are grouped rather
than issued individually.  Small host-packed constants ride the sync-engine
HWDGE queue.

Phase B on packed coordinates: row tiles of 128; only pair blocks (a,a) and
(a,a-1) exist (a batch spans at most 2 tiles).  Per block the PE does ONE
rank-1 broadcast (ones16.T @ v16row); DVE adds a host-precomputed fp16
additive mask (triangle + batch-boundary + padding, NEG=-30000 kills
invalid pairs through the exp); ACT applies one biased Exp per tile pair
(bias column = u + c from small [2,128]->[128,2] PE transposes, f32) and
grouped Ln(1+x) with fused row-sum accumulation.  The label-1 diagonal is
two masked column sums of raw u,v (negated masks, summed into the final
scalar by PSUM-accumulated matmuls).  A dummy Exp at kernel start preloads
the ACT table (saves ~1.5us off the tail); all ACT functions share the
single natural_log_exp_and_others table so it is loaded exactly once.

The host adds the 8 per-core partial sums, subtracts the c-terms, and
divides by the exact n_valid from the mask.  The program is compiled per
(R_data, R_tile) and cached; any mask/lengths distribution works.
"""

import glob
import json
import os
import shutil
import sys
import tempfile

if "/opt/trn_rl_repo" not in sys.path:
    sys.path.insert(0, "/opt/trn_rl_repo")


def _force_combined_act_table():
    """Point walrus at an act_info.json holding only natural_log_exp_and_others
    (contains exp+ln), so every ACTIVATE shares one table."""
    if os.environ.get("BASS_ACT_ROOT_JSON_PATH"):
        return
    from neuronxcc.driver.Job import Job  # type: ignore

    pwp = None
    for cand in glob.glob(os.path.join(Job.getPackageDir(), "pwp", "pwp_bin_*")):
        if os.path.exists(os.path.join(cand, "act_info.json")):
            pwp = cand
            break
    if pwp is None:
        return
    info = json.load(open(os.path.join(pwp, "act_info.json")))
    keep = [t for t in info.get("act_func_sets", [])
            if t.get("name") == "natural_log_exp_and_others"]
    if not keep:
        return
    out_dir = os.path.join(tempfile.gettempdir(), "dli_act_combined")
    os.makedirs(out_dir, exist_ok=True)
    for t in keep:
        for k in info.get("pwp_file_keys", []):
            f = t.get(k)
            src = os.path.join(pwp, f) if f else None
            if src and os.path.exists(src):
                dst = os.path.join(out_dir, f)
                if not os.path.exists(dst):
                    shutil.copy(src, dst)
    info = dict(info)
    info["act_func_sets"] = keep
    with open(os.path.join(out_dir, "act_info.json"), "w") as f:
        json.dump(info, f)
    os.environ["BASS_ACT_ROOT_JSON_PATH"] = os.path.join(out_dir, "act_info.json")


_force_combined_act_table()

from contextlib import ExitStack

import numpy as np

import concourse.bacc as bacc
import concourse.bass as bass
import concourse.hw_specs as hw_specs
import concourse.mybir as mybir
import concourse.tile as tile

# Make bass's act-table placement agree with the trimmed act_info.json walrus
# sees: only the combined exp+ln table exists, so every ACTIVATE maps to
# act_func_set_id 0 and the table is loaded exactly once.
_orig_get_act_tables = hw_specs.get_activation_tables


def _combined_act_tables(module_arch):
    tabs = _orig_get_act_tables(module_arch)
    kept = {k: v for k, v in tabs.items() if k == "natural_log_exp_and_others"}
    return kept if kept and os.environ.get("BASS_ACT_ROOT_JSON_PATH") else tabs


hw_specs.get_activation_tables = _combined_act_tables
bacc.get_activation_tables = _combined_act_tables

# Cheaper kernel teardown: drain + one all-engine barrier + sem clear. The
# stock epilogue adds a second all-engine barrier after the clear; engines
# that pass the first barrier only run their terminal branch, and the next
# execution starts only after every engine (incl. the clearing one) halts,
# so the second barrier only adds ~4us of EVSEM latency.
from concourse.vector_clock import ScopedClock as _ScopedClock


def _cheap_drain_and_barrier(self, tick_clock, wait_clock):
    drain_inst = self.nc.sync.drain()
    wait_clock.add_sem_waits(
        drain_inst.ins, _ScopedClock({None: tick_clock.global_clock})
    )
    self.nc.all_engine_barrier()
    popped = self.nc._tile_sem_poison_stack.pop()
    assert popped is self._sem_poison
    self.nc.clear_and_free_semaphores(list(self.sems.allocated().values()))


tile.TileContext._drain_and_barrier = _cheap_drain_and_barrier

F32 = mybir.dt.float32
BF16 = mybir.dt.bfloat16
FP16 = mybir.dt.float16
ALU = mybir.AluOpType
ACTF = mybir.ActivationFunctionType
AXX = mybir.AxisListType.X

BSZ, L, D = 64, 128, 2048
N_CORES = 8
NB = BSZ // N_CORES  # batches per core
NCH = D // 128  # 16 contraction chunks
NEG = -30000.0  # additive suppression; exp(NEG + anything sane) == 0 in f32
ABID = 512.0  # batch-id suppression scale; ABID*bid is fp16-exact for bid<=7


def build_program(R_data, R_tile):
    """R_data = packed row count (mult of 32, >=256); R_tile = mult of 128."""
    T = R_tile // 128
    NBLK = 2 * T - 1
    W32 = 2 * NCH + 3 * T + 3  # wvu | auxc | mcols | ones_col | ident2
    nc = bacc.Bacc("TRN2", target_bir_lowering=False, debug=False, num_devices=1)

    # partition-major: enc[p, ch*R + r] = encT[128*ch + p, r] -> one DMA
    # descriptor per partition per group (SWDGE ucode gen is the pacer)
    enc = nc.dram_tensor("enc", [128, NCH * R_data], F32, kind="ExternalInput").ap()
    sf32 = nc.dram_tensor("sf32", [128, W32], F32, kind="ExternalInput").ap()
    # per-block additive masks (tri + batch-boundary + padding in one):
    # block order (0,0), (1,1), (1,0), (2,2), (2,1), ...
    sf16m = nc.dram_tensor("sf16m", [128, NBLK * 128], FP16, kind="ExternalInput").ap()
    sf16r = nc.dram_tensor("sf16r", [1, 128], FP16, kind="ExternalInput").ap()
    out = nc.dram_tensor("out", [1, 1], F32, kind="ExternalOutput").ap()

    # psum column slices of <=512 f32 (one 2KB bank each); only R_data cols
    slices = [(s, min(s + 512, R_data)) for s in range(0, R_data, 512)]

    GROUPS = [1, 1, 2, 4, 4, 4] if NCH == 16 else [NCH]

    with tile.TileContext(nc) as tc, ExitStack() as ctx:
        consts = ctx.enter_context(tc.tile_pool(name="consts", bufs=1))
        accs = ctx.enter_context(tc.tile_pool(name="accs", bufs=1))
        enc_pool = ctx.enter_context(tc.tile_pool(name="enc", bufs=1))
        junk_pool = ctx.enter_context(tc.tile_pool(name="junk", bufs=2))
        psum_uv_pool = ctx.enter_context(tc.tile_pool(name="psuv", bufs=1, space="PSUM"))
        psum_d_pool = ctx.enter_context(tc.tile_pool(name="psd", bufs=2, space="PSUM"))
        psum_misc = ctx.enter_context(tc.tile_pool(name="psm", bufs=1, space="PSUM"))

        # ---- ACT table preload: dummy exp so the ~1.5us table load happens
        # during the DMA phase rather than before the first real Exp ----
        dummy = consts.tile([1, 1], F32)
        nc.gpsimd.memset(dummy[:], 0.0)
        dummy2 = consts.tile([1, 1], F32)
        nc.scalar.activation(dummy2[:], dummy[:], ACTF.Exp)

        # ---- enc group DMAs (single sync queue; each DMA_DIRECT2D trigger
        # costs ~750ns serially on the engine), constants interleaved ----
        encb, ch_group = [], {}
        ch0 = 0
        sf32_sb = consts.tile([128, W32], F32)
        sf16m_sb = consts.tile([128, NBLK * 128], FP16)
        sf16r_sb = consts.tile([1, 128], FP16)
        for g, cpg in enumerate(GROUPS):
            tb = enc_pool.tile([128, cpg * R_data], BF16, name=f"encb{g}", tag=f"encb{g}")
            nc.gpsimd.dma_start(
                tb[:], enc[:, ch0 * R_data : (ch0 + cpg) * R_data])
            encb.append(tb)
            for cl in range(cpg):
                ch_group[ch0 + cl] = (g, cl)
            ch0 += cpg
            if g == 0:
                nc.sync.dma_start(sf32_sb[:], sf32[:])
                nc.sync.dma_start(sf16m_sb[:], sf16m[:])
                nc.sync.dma_start(sf16r_sb[:], sf16r[:])

        auxc_sb = sf32_sb[:, 2 * NCH : 2 * NCH + T]           # u bias addend cols
        mcols = sf32_sb[:, 2 * NCH + T : 2 * NCH + 3 * T]     # negated diag masks
        ones_col = sf32_sb[:, 2 * NCH + 3 * T : 2 * NCH + 3 * T + 1]
        ident2 = sf32_sb[0:2, 2 * NCH + 3 * T + 1 : 2 * NCH + 3 * T + 3]
        ones16 = sf16r_sb[0:1, 0:128]

        wvu_bf = consts.tile([128, 2 * NCH], BF16)
        nc.vector.tensor_copy(wvu_bf[:], sf32_sb[:, 0 : 2 * NCH])

        # ---- v,u dots on the PE (v row 0 so it is partition-0 readable) ----
        psum_uv = []
        for s, (c0, c1) in enumerate(slices):
            psum_uv.append(
                psum_uv_pool.tile([2, c1 - c0], F32, tag=f"uv{s}", name=f"uv{s}"))
        for ch in range(NCH):
            g, cl = ch_group[ch]
            for s, (c0, c1) in enumerate(slices):
                nc.tensor.matmul(
                    psum_uv[s][:], lhsT=wvu_bf[:, 2 * ch : 2 * ch + 2],
                    rhs=encb[g][:, cl * R_data + c0 : cl * R_data + c1],
                    start=(ch == 0), stop=(ch == NCH - 1),
                )

        # ---- extraction: v row directly; u columns (for the exp bias) via
        # small [2,128]->[128,2] PE transposes. A few junk matmuls keep the
        # PE p-state warm across the DVE extraction gap. ----
        v16 = accs.tile([1, R_tile], FP16)
        if R_data < R_tile:
            nc.vector.memset(v16[:, R_data:R_tile], 0.0)
        for s, (c0, c1) in enumerate(slices):
            nc.vector.tensor_copy(v16[:, c0:c1], psum_uv[s][0:1, :])
        uv_sb = accs.tile([2, R_tile], F32)
        if R_data < R_tile:
            nc.vector.memset(uv_sb[:, R_data:R_tile], 0.0)
        for s, (c0, c1) in enumerate(slices):
            nc.vector.tensor_copy(uv_sb[:, c0:c1], psum_uv[s][:])

        psum_j = psum_misc.tile([128, 128], F32, tag="junkp", name="junkp")
        for _ in range(3):
            nc.tensor.matmul(psum_j[:], lhsT=sf16m_sb[:, 0:128],
                             rhs=sf16m_sb[:, 0:128], start=True, stop=True)

        psum_t = psum_misc.tile([128, 2 * T], F32, tag="uc", name="uc")
        ubias = accs.tile([128, T], F32)
        for a in range(T):
            nc.tensor.matmul(
                psum_t[:, 2 * a : 2 * a + 2],
                lhsT=uv_sb[0:2, 128 * a : 128 * (a + 1)],
                rhs=ident2, is_transpose=True,
            )
            nc.vector.tensor_add(ubias[:, a : a + 1], psum_t[:, 2 * a + 1 : 2 * a + 2],
                                 auxc_sb[:, a : a + 1])

        # ---- phase B: per tile a, blocks (a,a)+(a,a-1) share one psum pair;
        # PE does only the v broadcast; DVE adds the host mask from psum;
        # ACT does one biased Exp per tile and grouped Ln(1+x) row-sums ----
        nblk_of = [1 if a == 0 else 2 for a in range(T)]
        ln_groups = []  # list of (exg_tile, [tiles]) widths 384/512/...
        RS = accs.tile([128, max(1, (T + 1) // 2)], F32)
        # pair tiles into ln groups of total width <= 512
        grp, wsum, groups = [], 0, []
        for a in range(T):
            w = 128 * nblk_of[a]
            if wsum + w > 512 and grp:
                groups.append(grp)
                grp, wsum = [], 0
            grp.append(a)
            wsum += w
        if grp:
            groups.append(grp)
        exg_tiles = []
        exg_of = {}
        for gi, g in enumerate(groups):
            wtot = sum(128 * nblk_of[a] for a in g)
            tgl = accs.tile([128, wtot], F32, tag=f"exg{gi}", name=f"exg{gi}")
            exg_tiles.append(tgl)
            off = 0
            for a in g:
                exg_of[a] = (gi, off)
                off += 128 * nblk_of[a]

        # label-1 diagonal in column space (DVE; overlaps phase B below)
        uvm = accs.tile([128, 2 * T], F32)
        nc.vector.tensor_mul(uvm[:], psum_t[:], mcols)
        dsumc = accs.tile([128, 1], F32)
        nc.vector.reduce_sum(dsumc[:], uvm[:], axis=AXX)

        blk_idx = 0
        for a in range(T):
            nb = nblk_of[a]
            pd = psum_d_pool.tile([128, 128 * nb], F32, tag="pd")
            for q in range(nb):
                b = a - q
                nc.tensor.matmul(
                    pd[:, 128 * q : 128 * (q + 1)], lhsT=ones16,
                    rhs=v16[:, 128 * b : 128 * (b + 1)], start=True, stop=True)
            exd = junk_pool.tile([128, 128 * nb], F32, tag="exd")
            nc.vector.tensor_add(
                exd[:], pd[:], sf16m_sb[:, 128 * blk_idx : 128 * (blk_idx + nb)])
            blk_idx += nb
            gi, off = exg_of[a]
            nc.scalar.activation(
                exg_tiles[gi][:, off : off + 128 * nb], exd[:], ACTF.Exp,
                bias=ubias[:, a : a + 1],
            )
            if a == groups[len(ln_groups)][-1]:
                gi2 = len(ln_groups)
                sp = junk_pool.tile(list(exg_tiles[gi2].shape), F32, tag="sp")
                nc.scalar.activation(sp[:], exg_tiles[gi2][:], ACTF.Ln, bias=1.0,
                                     accum_out=RS[:, gi2 : gi2 + 1])
                ln_groups.append(gi2)

        # ---- final reduction ----
        accA = accs.tile([128, 1], F32)
        nc.vector.reduce_sum(accA[:], RS[:, 0 : len(groups)], axis=AXX)
        nc.vector.tensor_add(accA[:], accA[:], dsumc[:])
        psum_s = psum_misc.tile([1, 1], F32, tag="psm", name="psm")
        nc.tensor.matmul(psum_s[:], lhsT=accA[:], rhs=ones_col,
                         start=True, stop=True)
        out_t = accs.tile([1, 1], F32)
        nc.vector.tensor_copy(out_t[:], psum_s[:])
        nc.sync.dma_start(out[:], out_t[:])

    nc.compile()
    return nc


_NC_CACHE = {}
_LAST_KEY = None


def _get_nc(key=None):
    global _LAST_KEY
    if key is None:
        key = _LAST_KEY if _LAST_KEY is not None else (544, 640)
    if key not in _NC_CACHE:
        _NC_CACHE[key] = build_program(*key)
    _LAST_KEY = key
    return _NC_CACHE[key]


def _prep(encoder_output, mask, W, b):
    """Host-side prep: length-balanced shard + packed transposed layout."""
    enc = np.asarray(encoder_output, dtype=np.float32)
    W = np.asarray(W, dtype=np.float32)
    b = np.asarray(b, dtype=np.float32).reshape(2)
    mask = np.asarray(mask)
    c = float(b[1] - b[0])
    lens = mask.astype(np.int64).sum(axis=1)  # [BSZ]

    # greedy length-balance into N_CORES groups of NB
    order = np.argsort(-lens, kind="stable")
    assign = [[] for _ in range(N_CORES)]
    loads = [0] * N_CORES
    for bidx in order:
        for cid in sorted(range(N_CORES), key=lambda q: (loads[q], q)):
            if len(assign[cid]) < NB:
                assign[cid].append(int(bidx))
                loads[cid] += int(lens[bidx])
                break
    R_data = max(256, -(-max(loads) // 64) * 64)
    R_tile = -(-R_data // 128) * 128
    T = R_tile // 128
    NBLK = 2 * T - 1
    W32 = 2 * NCH + 3 * T + 3

    wd = np.stack([W[D:, 1] - W[D:, 0], W[:D, 1] - W[:D, 0]])  # [v-w, u-w]

    maps = []
    for cid in range(N_CORES):
        buf = np.zeros((R_data, D), dtype=np.float32)
        addu = np.full(R_tile, NEG, dtype=np.float32)   # u-side bias addend
        m1n = np.zeros(R_tile, dtype=np.float32)
        m2n = np.zeros(R_tile, dtype=np.float32)
        bid = np.full(R_tile, -1, dtype=np.int64)       # batch id per row
        off = 0
        for beta, bidx in enumerate(assign[cid]):
            ln_ = int(lens[bidx])
            buf[off : off + ln_] = enc[bidx, :ln_]
            addu[off : off + ln_] = c
            bid[off : off + ln_] = beta
            m1n[off + 1 : off + ln_] = -1.0        # m1 negated (not first row)
            m2n[off : off + ln_ - 1] = -1.0        # m2 negated (not last row)
            off += ln_
        # per-block additive masks: 0 iff same batch AND global k < j
        gj = np.arange(R_tile)
        valid = (bid[:, None] == bid[None, :]) & (bid[:, None] >= 0) & (
            gj[None, :] < gj[:, None])
        sf16m = np.full((128, NBLK * 128), NEG, dtype=np.float16)
        blk = 0
        for a in range(T):
            for q in range(1 if a == 0 else 2):
                bt = a - q
                sub = valid[128 * a : 128 * (a + 1), 128 * bt : 128 * (bt + 1)]
                sf16m[:, 128 * blk : 128 * (blk + 1)] = np.where(sub, 0.0, NEG)
                blk += 1
        sf32 = np.zeros((128, W32), dtype=np.float32)
        sf32[:, 0 : 2 * NCH] = wd.T.reshape(NCH, 128, 2).transpose(
            1, 0, 2).reshape(128, 2 * NCH)
        sf32[:, 2 * NCH : 2 * NCH + T] = addu.reshape(T, 128).T
        # mcols matches psum_t layout: even col = v (m2n), odd col = u (m1n)
        mc = np.stack([m2n.reshape(T, 128).T, m1n.reshape(T, 128).T], axis=2)
        sf32[:, 2 * NCH + T : 2 * NCH + 3 * T] = mc.reshape(128, 2 * T)
        sf32[:, 2 * NCH + 3 * T] = 1.0             # ones_col
        sf32[0, 2 * NCH + 3 * T + 1] = 1.0         # ident2
        sf32[1, 2 * NCH + 3 * T + 2] = 1.0
        sf16r = np.ones((1, 128), dtype=np.float16)
        maps.append(
            {
                "enc": np.ascontiguousarray(
                    buf.T.reshape(NCH, 128, R_data).transpose(1, 0, 2)
                    .reshape(128, NCH * R_data)),
                "sf32": sf32,
                "sf16m": sf16m,
                "sf16r": sf16r,
            }
        )
    diag_c = float(c * (lens - 1).sum())
    n_valid = int((lens * (lens - 1) // 2).sum())
    return maps, diag_c, n_valid, (R_data, R_tile)


def kernel(encoder_output, mask, W, b, _run_kwargs=None):
    from concourse.bass_utils import run_bass_kernel_spmd

    maps, diag_c, n_valid, key = _prep(np.asarray(encoder_output), mask, W, b)
    nc = _get_nc(key)
    res = run_bass_kernel_spmd(nc, maps, core_ids=list(range(N_CORES)),
                               **(_run_kwargs or {}))
    total = float(sum(np.float64(r["out"][0, 0]) for r in res.results))
    total -= diag_c
    loss = total / max(n_valid, 1)
    out = np.array(loss, dtype=np.float32)
    if _run_kwargs is not None:
        return out, res
    return out


# revision 26
# speedup vs baseline: 1.0660x; 1.0339x over previous
"""Trainium2 Bass kernel for the DLI (dialogue-turn ordering) loss — v8.

Math (exact reduction of the reference):
  With 2 classes, NLL(label y) = softplus(l_{1-y} - l_y).
  u[b,j] = enc[b,j] @ (W[:D,1]-W[:D,0]),
  v[b,k] = enc[b,k] @ (W[D:,1]-W[D:,0]),
  c      = b[1]-b[0],  d[b,j,k] = u[b,j] + v[b,k] + c
  label = 1 iff k == j-1; valid pairs: k < j < len_b;  softplus(-d) = softplus(d) - d
  =>  sum_nll = sum_{valid} softplus(d) - sum_{b, 1<=j<len_b} d[b,j,j-1]
  loss = sum_nll / max(n_valid, 1)

Ragged packing + PE dots (the arch is ragged_sequence — only rows j < len_b
matter, ~half of the 64MB input).  The host length-balances the 64 batches
into 8 cores of 8, packs each core's valid rows contiguously and ships them
TRANSPOSED: encT [D=2048, R_data] f32, R_data = rows rounded to 64 (576 for
the expected data; rows 256B-aligned for DMA).  HBM traffic drops ~45%, and
the transposed layout moves the u,v dots from DVE/ACT (the v1 bottleneck:
30us busy each) onto the idle TensorEngine:

    psum_uv[2, R] += wvuT_chunk[128, 2].T @ encT_chunk[128, R]   (16 chunks,
    bf16, PSUM f32 accumulation; v in row 0 so it is partition-0 readable)

enc arrives via SWDGE casting DMAs (HBM reads stay f32; bf16 lands in SBUF,
no staging or DVE casts), in groups of (1,1,2,4,4,4) chunks — the first
groups small so the dot pipeline starts as early as possible; each DMA
trigger costs ~750ns serially on its engine, so chunks are grouped rather
than issued individually.  Small host-packed constants ride the sync-engine
HWDGE queue.

Phase B on packed coordinates: row tiles of 128; only pair blocks (a,a) and
(a,a-1) exist (a batch spans at most 2 tiles).  Per block the PE does ONE
rank-1 broadcast (ones16.T @ v16row); DVE adds a host-precomputed fp16
additive mask (triangle + batch-boundary + padding, NEG=-30000 kills
invalid pairs through the exp); ACT applies one biased Exp per tile pair
(bias column = u + c from small [2,128]->[128,2] PE transposes, f32) and
grouped Ln(1+x) with fused row-sum accumulation.  The label-1 diagonal is
two masked column sums of raw u,v (negated masks, summed into the final
scalar by PSUM-accumulated matmuls).  A dummy Exp at kernel start preloads
the ACT table (saves ~1.5us off the tail); all ACT functions share the
single natural_log_exp_and_others table so it is loaded exactly once.

The host adds the 8 per-core partial sums, subtracts the c-terms, and
divides by the exact n_valid from the mask.  The program is compiled per
(R_data, R_tile) and cached; any mask/lengths distribution works.
"""

import glob
import json
import os
import shutil
import sys
import tempfile

if "/opt/trn_rl_repo" not in sys.path:
    sys.path.insert(0, "/opt/trn_rl_repo")


def _force_combined_act_table():
    """Point walrus at an act_info.json holding only natural_log_exp_and_others
    (contains exp+ln), so every ACTIVATE shares one table."""
    if os.environ.get("BASS_ACT_ROOT_JSON_PATH"):
        return
    from neuronxcc.driver.Job import Job  # type: ignore

    pwp = None
    for cand in glob.glob(os.path.join(Job.getPackageDir(), "pwp", "pwp_bin_*")):
        if os.path.exists(os.path.join(cand, "act_info.json")):
            pwp = cand
            break
    if pwp is None:
        return
    info = json.load(open(os.path.join(pwp, "act_info.json")))
    keep = [t for t in info.get("act_func_sets", [])
            if t.get("name") == "natural_log_exp_and_others"]
    if not keep:
        return
    out_dir = os.path.join(tempfile.gettempdir(), "dli_act_combined")
    os.makedirs(out_dir, exist_ok=True)
    for t in keep:
        for k in info.get("pwp_file_keys", []):
            f = t.get(k)
            src = os.path.join(pwp, f) if f else None
            if src and os.path.exists(src):
                dst = os.path.join(out_dir, f)
                if not os.path.exists(dst):
                    shutil.copy(src, dst)
    info = dict(info)
    info["act_func_sets"] = keep
    with open(os.path.join(out_dir, "act_info.json"), "w") as f:
        json.dump(info, f)
    os.environ["BASS_ACT_ROOT_JSON_PATH"] = os.path.join(out_dir, "act_info.json")


_force_combined_act_table()

from contextlib import ExitStack

import numpy as np

import concourse.bacc as bacc
import concourse.bass as bass
import concourse.hw_specs as hw_specs
import concourse.mybir as mybir
import concourse.tile as tile

# Make bass's act-table placement agree with the trimmed act_info.json walrus
# sees: only the combined exp+ln table exists, so every ACTIVATE maps to
# act_func_set_id 0 and the table is loaded exactly once.
_orig_get_act_tables = hw_specs.get_activation_tables


def _combined_act_tables(module_arch):
    tabs = _orig_get_act_tables(module_arch)
    kept = {k: v for k, v in tabs.items() if k == "natural_log_exp_and_others"}
    return kept if kept and os.environ.get("BASS_ACT_ROOT_JSON_PATH") else tabs


hw_specs.get_activation_tables = _combined_act_tables
bacc.get_activation_tables = _combined_act_tables

# Cheaper kernel teardown: drain + one all-engine barrier + sem clear. The
# stock epilogue adds a second all-engine barrier after the clear; engines
# that pass the first barrier only run their terminal branch, and the next
# execution starts only after every engine (incl. the clearing one) halts,
# so the second barrier only adds ~4us of EVSEM latency.
from concourse.vector_clock import ScopedClock as _ScopedClock


def _cheap_drain_and_barrier(self, tick_clock, wait_clock):
    drain_inst = self.nc.sync.drain()
    wait_clock.add_sem_waits(
        drain_inst.ins, _ScopedClock({None: tick_clock.global_clock})
    )
    self.nc.all_engine_barrier()
    popped = self.nc._tile_sem_poison_stack.pop()
    assert popped is self._sem_poison
    self.nc.clear_and_free_semaphores(list(self.sems.allocated().values()))


tile.TileContext._drain_and_barrier = _cheap_drain_and_barrier

F32 = mybir.dt.float32
BF16 = mybir.dt.bfloat16
FP16 = mybir.dt.float16
ALU = mybir.AluOpType
ACTF = mybir.ActivationFunctionType
AXX = mybir.AxisListType.X

BSZ, L, D = 64, 128, 2048
N_CORES = 8
NB = BSZ // N_CORES  # batches per core
NCH = D // 128  # 16 contraction chunks
NEG = -30000.0  # additive suppression; exp(NEG + anything sane) == 0 in f32
ABID = 512.0  # batch-id suppression scale; ABID*bid is fp16-exact for bid<=7


def build_program(R_data, R_tile):
    """R_data = packed row count (mult of 32, >=256); R_tile = mult of 128."""
    T = R_tile // 128
    NBLK = 2 * T - 1
    W32 = 2 * NCH + 3 * T + 3  # wvu | auxc | mcols | ones_col | ident2
    nc = bacc.Bacc("TRN2", target_bir_lowering=False, debug=False, num_devices=1,
                   enable_partition_id=False)

    enc = nc.dram_tensor("enc", [D, R_data], F32, kind="ExternalInput").ap()
    sf32 = nc.dram_tensor("sf32", [128, W32], F32, kind="ExternalInput").ap()
    # per-block additive masks (tri + batch-boundary + padding in one):
    # block order (0,0), (1,1), (1,0), (2,2), (2,1), ...
    sf16m = nc.dram_tensor("sf16m", [128, NBLK * 128], FP16, kind="ExternalInput").ap()
    sf16r = nc.dram_tensor("sf16r", [1, 128], FP16, kind="ExternalInput").ap()
    out = nc.dram_tensor("out", [1, 1], F32, kind="ExternalOutput").ap()

    # psum column slices of <=512 f32 (one 2KB bank each); only R_data cols
    slices = [(s, min(s + 512, R_data)) for s in range(0, R_data, 512)]

    GROUPS = [1, 1, 2, 4, 4, 4] if NCH == 16 else [NCH]

    with tile.TileContext(nc) as tc, ExitStack() as ctx:
        consts = ctx.enter_context(tc.tile_pool(name="consts", bufs=1))
        accs = ctx.enter_context(tc.tile_pool(name="accs", bufs=1))
        enc_pool = ctx.enter_context(tc.tile_pool(name="enc", bufs=1))
        junk_pool = ctx.enter_context(tc.tile_pool(name="junk", bufs=2))
        psum_uv_pool = ctx.enter_context(tc.tile_pool(name="psuv", bufs=1, space="PSUM"))
        psum_d_pool = ctx.enter_context(tc.tile_pool(name="psd", bufs=2, space="PSUM"))
        psum_misc = ctx.enter_context(tc.tile_pool(name="psm", bufs=1, space="PSUM"))

        # ---- ACT table preload: dummy exp so the ~1.5us table load happens
        # during the DMA phase rather than before the first real Exp ----
        dummy = consts.tile([1, 1], F32)
        nc.gpsimd.memset(dummy[:], 0.0)
        dummy2 = consts.tile([1, 1], F32)
        nc.scalar.activation(dummy2[:], dummy[:], ACTF.Exp)

        # ---- enc group DMAs (single sync queue; each DMA_DIRECT2D trigger
        # costs ~750ns serially on the engine), constants interleaved ----
        encb, ch_group = [], {}
        ch0 = 0
        sf32_sb = consts.tile([128, W32], F32)
        sf16m_sb = consts.tile([128, NBLK * 128], FP16)
        sf16r_sb = consts.tile([1, 128], FP16)
        for g, cpg in enumerate(GROUPS):
            tb = enc_pool.tile([128, cpg * R_data], BF16, name=f"encb{g}", tag=f"encb{g}")
            nc.gpsimd.dma_start(
                tb[:].rearrange("p (c r) -> p c r", c=cpg),
                enc[128 * ch0 : 128 * (ch0 + cpg), :].rearrange("(c p) r -> p c r", p=128),
            )
            encb.append(tb)
            for cl in range(cpg):
                ch_group[ch0 + cl] = (g, cl)
            ch0 += cpg
            if g == 0:
                nc.sync.dma_start(sf32_sb[:], sf32[:])
                nc.sync.dma_start(sf16m_sb[:], sf16m[:])
                nc.sync.dma_start(sf16r_sb[:], sf16r[:])

        auxc_sb = sf32_sb[:, 2 * NCH : 2 * NCH + T]           # u bias addend cols
        mcols = sf32_sb[:, 2 * NCH + T : 2 * NCH + 3 * T]     # negated diag masks
        ones_col = sf32_sb[:, 2 * NCH + 3 * T : 2 * NCH + 3 * T + 1]
        ident2 = sf32_sb[0:2, 2 * NCH + 3 * T + 1 : 2 * NCH + 3 * T + 3]
        ones16 = sf16r_sb[0:1, 0:128]

        wvu_bf = consts.tile([128, 2 * NCH], BF16)
        nc.vector.tensor_copy(wvu_bf[:], sf32_sb[:, 0 : 2 * NCH])

        # ---- v,u dots on the PE (v row 0 so it is partition-0 readable) ----
        psum_uv = []
        for s, (c0, c1) in enumerate(slices):
            psum_uv.append(
                psum_uv_pool.tile([2, c1 - c0], F32, tag=f"uv{s}", name=f"uv{s}"))
        for ch in range(NCH):
            g, cl = ch_group[ch]
            for s, (c0, c1) in enumerate(slices):
                nc.tensor.matmul(
                    psum_uv[s][:], lhsT=wvu_bf[:, 2 * ch : 2 * ch + 2],
                    rhs=encb[g][:, cl * R_data + c0 : cl * R_data + c1],
                    start=(ch == 0), stop=(ch == NCH - 1),
                )

        # ---- extraction: v row directly; u columns (for the exp bias) via
        # small [2,128]->[128,2] PE transposes. A few junk matmuls keep the
        # PE p-state warm across the DVE extraction gap. ----
        v16 = accs.tile([1, R_tile], FP16)
        if R_data < R_tile:
            nc.vector.memset(v16[:, R_data:R_tile], 0.0)
        for s, (c0, c1) in enumerate(slices):
            nc.vector.tensor_copy(v16[:, c0:c1], psum_uv[s][0:1, :])
        uv_sb = accs.tile([2, R_tile], F32)
        if R_data < R_tile:
            nc.vector.memset(uv_sb[:, R_data:R_tile], 0.0)
        for s, (c0, c1) in enumerate(slices):
            nc.vector.tensor_copy(uv_sb[:, c0:c1], psum_uv[s][:])

        psum_j = psum_misc.tile([128, 128], F32, tag="junkp", name="junkp")
        for _ in range(3):
            nc.tensor.matmul(psum_j[:], lhsT=sf16m_sb[:, 0:128],
                             rhs=sf16m_sb[:, 0:128], start=True, stop=True)

        psum_t = psum_misc.tile([128, 2 * T], F32, tag="uc", name="uc")
        ubias = accs.tile([128, T], F32)
        for a in range(T):
            nc.tensor.matmul(
                psum_t[:, 2 * a : 2 * a + 2],
                lhsT=uv_sb[0:2, 128 * a : 128 * (a + 1)],
                rhs=ident2, is_transpose=True,
            )
            nc.vector.tensor_add(ubias[:, a : a + 1], psum_t[:, 2 * a + 1 : 2 * a + 2],
                                 auxc_sb[:, a : a + 1])

        # ---- phase B: per tile a, blocks (a,a)+(a,a-1) share one psum pair;
        # PE does only the v broadcast; DVE adds the host mask from psum;
        # ACT does one biased Exp per tile and grouped Ln(1+x) row-sums ----
        nblk_of = [1 if a == 0 else 2 for a in range(T)]
        ln_groups = []  # list of (exg_tile, [tiles]) widths 384/512/...
        RS = accs.tile([128, max(1, (T + 1) // 2)], F32)
        # pair tiles into ln groups of total width <= 512
        grp, wsum, groups = [], 0, []
        for a in range(T):
            w = 128 * nblk_of[a]
            if wsum + w > 512 and grp:
                groups.append(grp)
                grp, wsum = [], 0
            grp.append(a)
            wsum += w
        if grp:
            groups.append(grp)
        exg_tiles = []
        exg_of = {}
        for gi, g in enumerate(groups):
            wtot = sum(128 * nblk_of[a] for a in g)
            tgl = accs.tile([128, wtot], F32, tag=f"exg{gi}", name=f"exg{gi}")
            exg_tiles.append(tgl)
            off = 0
            for a in g:
                exg_of[a] = (gi, off)
                off += 128 * nblk_of[a]

        # label-1 diagonal in column space (DVE; overlaps phase B below)
        uvm = accs.tile([128, 2 * T], F32)
        nc.vector.tensor_mul(uvm[:], psum_t[:], mcols)
        dsumc = accs.tile([128, 1], F32)
        nc.vector.reduce_sum(dsumc[:], uvm[:], axis=AXX)

        blk_idx = 0
        for a in range(T):
            nb = nblk_of[a]
            pd = psum_d_pool.tile([128, 128 * nb], F32, tag="pd")
            for q in range(nb):
                b = a - q
                nc.tensor.matmul(
                    pd[:, 128 * q : 128 * (q + 1)], lhsT=ones16,
                    rhs=v16[:, 128 * b : 128 * (b + 1)], start=True, stop=True)
            exd = junk_pool.tile([128, 128 * nb], F32, tag="exd")
            nc.vector.tensor_add(
                exd[:], pd[:], sf16m_sb[:, 128 * blk_idx : 128 * (blk_idx + nb)])
            blk_idx += nb
            gi, off = exg_of[a]
            nc.scalar.activation(
                exg_tiles[gi][:, off : off + 128 * nb], exd[:], ACTF.Exp,
                bias=ubias[:, a : a + 1],
            )
            if a == groups[len(ln_groups)][-1]:
                gi2 = len(ln_groups)
                sp = junk_pool.tile(list(exg_tiles[gi2].shape), F32, tag="sp")
                nc.scalar.activation(sp[:], exg_tiles[gi2][:], ACTF.Ln, bias=1.0,
                                     accum_out=RS[:, gi2 : gi2 + 1])
                ln_groups.append(gi2)

        # ---- final reduction ----
        accA = accs.tile([128, 1], F32)
        nc.vector.reduce_sum(accA[:], RS[:, 0 : len(groups)], axis=AXX)
        nc.vector.tensor_add(accA[:], accA[:], dsumc[:])
        psum_s = psum_misc.tile([1, 1], F32, tag="psm", name="psm")
        nc.tensor.matmul(psum_s[:], lhsT=accA[:], rhs=ones_col,
                         start=True, stop=True)
        out_t = accs.tile([1, 1], F32)
        nc.vector.tensor_copy(out_t[:], psum_s[:])
        nc.sync.dma_start(out[:], out_t[:])

    nc.compile()
    return nc


_NC_CACHE = {}
_LAST_KEY = None


def _get_nc(key=None):
    global _LAST_KEY
    if key is None:
        key = _LAST_KEY if _LAST_KEY is not None else (544, 640)
    if key not in _NC_CACHE:
        _NC_CACHE[key] = build_program(*key)
    _LAST_KEY = key
    return _NC_CACHE[key]


def _prep(encoder_output, mask, W, b):
    """Host-side prep: length-balanced shard + packed transposed layout."""
    enc = np.asarray(encoder_output, dtype=np.float32)
    W = np.asarray(W, dtype=np.float32)
    b = np.asarray(b, dtype=np.float32).reshape(2)
    mask = np.asarray(mask)
    c = float(b[1] - b[0])
    lens = mask.astype(np.int64).sum(axis=1)  # [BSZ]

    # greedy length-balance into N_CORES groups of NB
    order = np.argsort(-lens, kind="stable")
    assign = [[] for _ in range(N_CORES)]
    loads = [0] * N_CORES
    for bidx in order:
        for cid in sorted(range(N_CORES), key=lambda q: (loads[q], q)):
            if len(assign[cid]) < NB:
                assign[cid].append(int(bidx))
                loads[cid] += int(lens[bidx])
                break
    R_data = max(256, -(-max(loads) // 64) * 64)
    R_tile = -(-R_data // 128) * 128
    T = R_tile // 128
    NBLK = 2 * T - 1
    W32 = 2 * NCH + 3 * T + 3

    wd = np.stack([W[D:, 1] - W[D:, 0], W[:D, 1] - W[:D, 0]])  # [v-w, u-w]

    maps = []
    for cid in range(N_CORES):
        buf = np.zeros((R_data, D), dtype=np.float32)
        addu = np.full(R_tile, NEG, dtype=np.float32)   # u-side bias addend
        m1n = np.zeros(R_tile, dtype=np.float32)
        m2n = np.zeros(R_tile, dtype=np.float32)
        bid = np.full(R_tile, -1, dtype=np.int64)       # batch id per row
        off = 0
        for beta, bidx in enumerate(assign[cid]):
            ln_ = int(lens[bidx])
            buf[off : off + ln_] = enc[bidx, :ln_]
            addu[off : off + ln_] = c
            bid[off : off + ln_] = beta
            m1n[off + 1 : off + ln_] = -1.0        # m1 negated (not first row)
            m2n[off : off + ln_ - 1] = -1.0        # m2 negated (not last row)
            off += ln_
        # per-block additive masks: 0 iff same batch AND global k < j
        gj = np.arange(R_tile)
        valid = (bid[:, None] == bid[None, :]) & (bid[:, None] >= 0) & (
            gj[None, :] < gj[:, None])
        sf16m = np.full((128, NBLK * 128), NEG, dtype=np.float16)
        blk = 0
        for a in range(T):
            for q in range(1 if a == 0 else 2):
                bt = a - q
                sub = valid[128 * a : 128 * (a + 1), 128 * bt : 128 * (bt + 1)]
                sf16m[:, 128 * blk : 128 * (blk + 1)] = np.where(sub, 0.0, NEG)
                blk += 1
        sf32 = np.zeros((128, W32), dtype=np.float32)
        sf32[:, 0 : 2 * NCH] = wd.T.reshape(NCH, 128, 2).transpose(
            1, 0, 2).reshape(128, 2 * NCH)
        sf32[:, 2 * NCH : 2 * NCH + T] = addu.reshape(T, 128).T
        # mcols matches psum_t layout: even col = v (m2n), odd col = u (m1n)
        mc = np.stack([m2n.reshape(T, 128).T, m1n.reshape(T, 128).T], axis=2)
        sf32[:, 2 * NCH + T : 2 * NCH + 3 * T] = mc.reshape(128, 2 * T)
        sf32[:, 2 * NCH + 3 * T] = 1.0             # ones_col
        sf32[0, 2 * NCH + 3 * T + 1] = 1.0         # ident2
        sf32[1, 2 * NCH + 3 * T + 2] = 1.0
        sf16r = np.ones((1, 128), dtype=np.float16)
        maps.append(
            {
                "enc": np.ascontiguousarray(buf.T),
                "sf32": sf32,
                "sf16m": sf16m,
                "sf16r": sf16r,
            }
        )
    diag_c = float(c * (lens - 1).sum())
    n_valid = int((lens * (lens - 1) // 2).sum())
    return maps, diag_c, n_valid, (R_data, R_tile)


def kernel(encoder_output, mask, W, b, _run_kwargs=None):
    from concourse.bass_utils import run_bass_kernel_spmd

    maps, diag_c, n_valid, key = _prep(np.asarray(encoder_output), mask, W, b)
    nc = _get_nc(key)
    res = run_bass_kernel_spmd(nc, maps, core_ids=list(range(N_CORES)),
                               **(_run_kwargs or {}))
    total = float(sum(np.float64(r["out"][0, 0]) for r in res.results))
    total -= diag_c
    loss = total / max(n_valid, 1)
    out = np.array(loss, dtype=np.float32)
    if _run_kwargs is not None:
        return out, res
    return out
